# revision 2
# baseline (speedup 1.0000x reference)
"""MoE expert-MLP (8 experts, top-2, capacity-factor 2) for 8 trn2 NeuronCores.

Strategy: expert-parallel. Host replicates the reference routing exactly
(cumsum capacity assignment, affinity re-normalization), gathers each
expert's assigned tokens into a compact padded buffer, and each core runs
one expert's GLU MLP (gate/up matmul -> silu*up -> down matmul) as a dense
fp32r kernel. The combine (aff-weighted sum over the token's top-k slots)
is linear, so it is done on host exactly as the reference does.

Device kernel per core (S=1024 compact token slots):
  phase 1: guT[f, t] accumulation over H, silu(gate)*up -> hT in SBUF
  phase 2: y[t, o]  accumulation over I -> DRAM
All matmuls in float32r (~1.5e-4 rel err, 4x faster than fp32 on PE).
"""

import math

import numpy as np

import concourse.bacc as bacc
import concourse.mybir as mybir
import concourse.tile as tile
from concourse.bass_utils import run_bass_kernel_spmd

E = 8
TOP_K = 2
H = 1024
I = 2816
T = 4096
CAPACITY_FACTOR = 2.0

S = 1024          # compact token slots per expert per launch (max observed load ~1002)
P = 128
HO = H // P       # 8 h-tiles
FI = I // P       # 22 f-tiles
NB = S // 512     # phase-1 token blocks
OT = H // 512     # phase-2 output col tiles

F32 = mybir.dt.float32
F32R = mybir.dt.float32r

_nc_cache = []


def _build_nc():
    nc = bacc.Bacc(None, target_bir_lowering=False)

    xt = nc.dram_tensor("xt", [H, S], F32R, kind="ExternalInput")        # tokens, transposed
    wg = nc.dram_tensor("wg", [FI, P, HO, P], F32R, kind="ExternalInput")  # gate, tiled
    wu = nc.dram_tensor("wu", [FI, P, HO, P], F32R, kind="ExternalInput")  # up, tiled
    wd = nc.dram_tensor("wd", [I, H], F32R, kind="ExternalInput")        # down, natural
    y = nc.dram_tensor("y", [S, H], F32, kind="ExternalOutput")

    with tile.TileContext(nc) as tc:
        with (
            tc.tile_pool(name="resident", bufs=1) as res_pool,
            tc.tile_pool(name="wstream", bufs=3) as w_pool,
            tc.tile_pool(name="act", bufs=3) as act_pool,
            tc.tile_pool(name="out", bufs=4) as out_pool,
        ):
            # resident: token activations (transposed) and intermediate hT
            xt_sb = res_pool.tile([P, HO, S], F32R, tag="xt")
            for h in range(HO):
                nc.sync.dma_start(
                    xt_sb[:, h, :],
                    xt[h * P:(h + 1) * P, :],
                )
            ht = res_pool.tile([P, FI, S], F32R, tag="ht")

            # ---- phase 1: guT tiles + silu*up -> hT ----
            with (
                tc.tile_pool(name="psg", bufs=2, space="PSUM") as psg_pool,
                tc.tile_pool(name="psu", bufs=2, space="PSUM") as psu_pool,
            ):
                for f in range(FI):
                    wg_f = w_pool.tile([P, HO, P], F32R, tag="wg")
                    nc.sync.dma_start(wg_f[:], wg[f])
                    wu_f = w_pool.tile([P, HO, P], F32R, tag="wu")
                    nc.sync.dma_start(wu_f[:], wu[f])
                    for tb in range(NB):
                        ps_g = psg_pool.tile([P, 512], F32, tag="psg")
                        ps_u = psu_pool.tile([P, 512], F32, tag="psu")
                        for h in range(HO):
                            nc.tensor.matmul(
                                ps_g[:],
                                wg_f[:, h],
                                xt_sb[:, h, tb * 512:(tb + 1) * 512],
                                start=(h == 0),
                                stop=(h == HO - 1),
                            )
                        for h in range(HO):
                            nc.tensor.matmul(
                                ps_u[:],
                                wu_f[:, h],
                                xt_sb[:, h, tb * 512:(tb + 1) * 512],
                                start=(h == 0),
                                stop=(h == HO - 1),
                            )
                        sil = act_pool.tile([P, 512], F32, tag="sil")
                        nc.scalar.activation(
                            sil[:], ps_g[:], mybir.ActivationFunctionType.Silu
                        )
                        nc.vector.tensor_tensor(
                            ht[:, f, tb * 512:(tb + 1) * 512],
                            sil[:],
                            ps_u[:],
                            mybir.AluOpType.mult,
                        )

            # ---- phase 2: y = hT.T @ wd ----
            with tc.tile_pool(name="pso", bufs=8, space="PSUM") as pso_pool:
                for half in range(NB):
                    pso = [
                        [pso_pool.tile([P, 512], F32, tag="pso",
                                       name=f"pso_{half}_{sub}_{o}")
                         for o in range(OT)]
                        for sub in range(4)
                    ]
                    for k in range(FI):
                        wd_k = w_pool.tile([P, H], F32R, tag="wd")
                        nc.sync.dma_start(wd_k[:], wd[k * P:(k + 1) * P, :])
                        for sub in range(4):
                            lh = ht[:, k, half * 512 + sub * P: half * 512 + (sub + 1) * P]
                            for o in range(OT):
                                nc.tensor.matmul(
                                    pso[sub][o][:],
                                    lh,
                                    wd_k[:, o * 512:(o + 1) * 512],
                                    start=(k == 0),
                                    stop=(k == FI - 1),
                                )
                    for sub in range(4):
                        for o in range(OT):
                            ot = out_pool.tile([P, 512], F32, tag="yo")
                            nc.vector.tensor_copy(ot[:], pso[sub][o][:])
                            nc.sync.dma_start(
                                y[half * 512 + sub * P: half * 512 + (sub + 1) * P,
                                  o * 512:(o + 1) * 512],
                                ot[:],
                            )

    nc.finalize()
    return nc


def _routing(expert_affinities, expert_index):
    """Exact numpy replica of the reference routing."""
    idx = np.asarray(expert_index).astype(np.int32)
    affin = np.asarray(expert_affinities).astype(np.float32)
    C = min(math.ceil(T * TOP_K * CAPACITY_FACTOR / E), T)

    mask = np.zeros((T, E), np.float32)
    for k in range(TOP_K):
        np.add.at(mask, (np.arange(T), idx[:, k]), 1.0)
    pos = np.cumsum(mask, axis=0, dtype=np.float32)
    mask = np.where(pos > C, 0.0, mask)
    aff = np.where(mask == 0, 0.0, affin)
    aff = aff / np.maximum(np.sum(np.abs(aff), axis=1, keepdims=True), 1e-12)
    offsets = np.arange(E, dtype=np.float32) * C
    pos_off = np.where(mask == 0, 0.0, pos + offsets[None, :])
    perm = np.take_along_axis(pos_off, idx, axis=1).astype(np.int32)  # 1-indexed
    vals = np.broadcast_to((np.arange(T, dtype=np.int32) + 1)[:, None], (T, TOP_K))
    assign = np.zeros(E * C + 1, np.int32)
    assign[perm.reshape(-1)] = vals.reshape(-1)
    assign = assign[1:].reshape(E, C)
    occupied = assign > 0
    assign0 = np.maximum(assign - 1, 0)
    perm0 = np.maximum(perm - 1, 0)
    aff_k = np.take_along_axis(aff, idx, axis=1)  # 0 for dropped pairs
    return C, occupied, assign0, perm0, aff_k


def kernel(hidden_states, expert_affinities, expert_index, w_gate_up, w_down):
    hid = np.ascontiguousarray(np.asarray(hidden_states, dtype=np.float32))
    wgu = np.asarray(w_gate_up, dtype=np.float32)
    wdn = np.asarray(w_down, dtype=np.float32)

    C, occupied, assign0, perm0, aff_k = _routing(expert_affinities, expert_index)

    # compact per-expert token lists (slot order preserved)
    c2s = [np.nonzero(occupied[e])[0] for e in range(E)]
    n_e = np.array([len(c) for c in c2s])
    chunks = max(1, int(math.ceil(n_e.max() / S)))

    # slot -> compact row lookup (unoccupied slots map to row 0; only read
    # with affinity weight 0, matching the reference's clamped drop reads)
    L = np.zeros(E * C, np.int64)
    for e in range(E):
        L[e * C + c2s[e]] = e * chunks * S + np.arange(n_e[e])

    if not _nc_cache:
        _nc_cache.append(_build_nc())
    nc = _nc_cache[0]

    # per-core static weight operands (reused across chunks)
    w_maps = []
    for e in range(E):
        wg_t = np.ascontiguousarray(
            wgu[e, :, :I].reshape(HO, P, FI, P).transpose(2, 1, 0, 3)
        )
        wu_t = np.ascontiguousarray(
            wgu[e, :, I:].reshape(HO, P, FI, P).transpose(2, 1, 0, 3)
        )
        wd_t = np.ascontiguousarray(wdn[e])
        w_maps.append({"wg": wg_t, "wu": wu_t, "wd": wd_t})

    ycomp = np.zeros((E * chunks * S, H), np.float32)
    for j in range(chunks):
        in_maps = []
        for e in range(E):
            tok = assign0[e][c2s[e]][j * S:(j + 1) * S]
            xt = np.zeros((H, S), np.float32)
            if len(tok):
                xt[:, :len(tok)] = hid[tok].T
            in_maps.append({"xt": xt, **w_maps[e]})
        res = run_bass_kernel_spmd(nc, in_maps, core_ids=list(range(E)))
        for e in range(E):
            lo = e * chunks * S + j * S
            n_rows = min(S, max(0, n_e[e] - j * S))
            if n_rows:
                ycomp[lo:lo + n_rows] = res.results[e]["y"][:n_rows]

    out = (ycomp[L[perm0[:, 0]]] * aff_k[:, 0, None]
           + ycomp[L[perm0[:, 1]]] * aff_k[:, 1, None])
    return out.astype(np.float32)
